# revision 5
# baseline (speedup 1.0000x reference)
"""DualScaleVQ Trainium2 kernel: 8-core SPMD, three device phases.

Phase 1 (data-parallel over N): coarse VQ scores s = 2*z@c.T - ||c||^2 with
fp16(hi)*fp16(hi) matmuls + an exact 2-row ||c||^2 bias chunk (error < ~0.09,
measured). Argmax via fused PSUM-spill/max + max_index, z_q via indirect
codebook gather, per-latent ||z||^2 (diagonal matmul) for the loss, plus an
ACT Relu-sum ambiguity detector and per-512-block coarse maxima.

Rescue (tiny, launched only if ambiguous rows exist): exact fp16 hi/lo
3-pass scores for ambiguous rows x candidate blocks; fixes the ~0.1% of
argmaxes the coarse pass cannot order (margin 0.3 >> max coarse error).

Host between phases does integer bookkeeping only (argsort/bincount/bucket).

Phase 2 (code-parallel over K): per-code embed sums/counts via segmented
one-hot matmuls, EMA codebook update, adjacency decay update from sparse
per-row pair counts (local_scatter + Exp LUT).
"""
import os
import sys

sys.path.insert(0, "/opt/trn_rl_repo")
os.environ.setdefault("MYCRO_LOCAL_CACHE", "1")

import numpy as np

from concourse import bass, bacc, mybir
import concourse.tile as tile
from concourse.bass_utils import run_bass_kernel_spmd

F16 = mybir.dt.float16
F32 = mybir.dt.float32
BF16 = mybir.dt.bfloat16
I16 = mybir.dt.int16
I32 = mybir.dt.int32
U32 = mybir.dt.uint32

C = 8                      # cores
N = 32768                  # latents
D = 512                    # 2 * latent_dim
KS, KM = 1024, 4096        # codebook sizes (syn, sem)
NLAT = N // C              # latents per core in phase 1
NT = NLAT // 128           # 128-latent tiles per core
RS, RM = KS // C, KM // C  # codebook rows per core in phase 2

DECAY = np.float32(0.99)
ONE_MINUS_DECAY = np.float32(1.0 - 0.99)
EPS = 1e-6
LN_ADJ_DECAY = float(np.log(np.float64(0.99)))
ADJ_Q = float(1.0 / (1.0 - 0.99))
MARGIN = 0.3               # coarse-score ambiguity margin (max observed ~0.09)
QS = 1024                  # local_scatter column quarter size

_PHASE1_NC = None
_RESCUE_NC = {}
_PHASE2_NC = {}
LAST_STATS = {}


def _run(nc, in_maps, core_ids, tag):
    res = run_bass_kernel_spmd(nc, in_maps, core_ids=core_ids)
    ns = res.exec_time_ns
    if ns is None and os.environ.get("VQ_MODEL_TIME", "0") == "1":
        # No NTFF profiling hook in this container build; fall back to the
        # instruction cost model (same model the Tile scheduler uses).
        if not hasattr(nc, "_modeled_ns"):
            from concourse.timeline_sim import TimelineSim
            nc._modeled_ns = TimelineSim(nc, no_exec=True).simulate()
        ns = nc._modeled_ns
    LAST_STATS[tag] = {"exec_time_ns": ns}
    return res


def _build_phase1():
    nc = bacc.Bacc(None, target_bir_lowering=False, debug=False)

    ins = {}
    for nm in ("zfT_hi", "zsT_hi"):
        ins[nm] = nc.dram_tensor(nm, [D, NLAT], F16, kind="ExternalInput")
    ins["csT_hi"] = nc.dram_tensor("csT_hi", [D, KS], F16, kind="ExternalInput")
    ins["cmT_hi"] = nc.dram_tensor("cmT_hi", [D, KM], F16, kind="ExternalInput")
    ins["c2s"] = nc.dram_tensor("c2s", [2, KS], F16, kind="ExternalInput")
    ins["c2m"] = nc.dram_tensor("c2m", [2, KM], F16, kind="ExternalInput")
    ins["cbs_nat"] = nc.dram_tensor("cbs_nat", [KS, D], F32, kind="ExternalInput")
    ins["cbm_nat"] = nc.dram_tensor("cbm_nat", [KM, D], F32, kind="ExternalInput")
    ins["ident"] = nc.dram_tensor("ident", [128, 128], F32, kind="ExternalInput")

    zq_syn = nc.dram_tensor("zq_syn", [NLAT, D], F32, kind="ExternalOutput")
    zq_sem = nc.dram_tensor("zq_sem", [NLAT, D], F32, kind="ExternalOutput")
    idx_syn = nc.dram_tensor("idx_syn", [NLAT, 1], I32, kind="ExternalOutput")
    idx_sem = nc.dram_tensor("idx_sem", [NLAT, 1], I32, kind="ExternalOutput")
    smax_syn = nc.dram_tensor("smax_syn", [128, NT], F32, kind="ExternalOutput")
    smax_sem = nc.dram_tensor("smax_sem", [128, NT], F32, kind="ExternalOutput")
    znorm_f = nc.dram_tensor("znorm_f", [128, NT], F32, kind="ExternalOutput")
    znorm_s = nc.dram_tensor("znorm_s", [128, NT], F32, kind="ExternalOutput")
    cnt_syn = nc.dram_tensor("cnt_syn", [128, NT], F32, kind="ExternalOutput")
    cnt_sem = nc.dram_tensor("cnt_sem", [128, NT], F32, kind="ExternalOutput")
    cmax_syn = nc.dram_tensor("cmax_syn", [128, NT * 2], F32, kind="ExternalOutput")
    cmax_sem = nc.dram_tensor("cmax_sem", [128, NT * 8], F32, kind="ExternalOutput")

    with tile.TileContext(nc) as tc:
        with tc.tile_pool(name="const", bufs=1) as cp, \
             tc.tile_pool(name="zin", bufs=3) as zp, \
             tc.tile_pool(name="srow", bufs=2) as sp, \
             tc.tile_pool(name="small", bufs=3) as mp, \
             tc.tile_pool(name="junk", bufs=2) as jp, \
             tc.tile_pool(name="zq", bufs=4) as qp, \
             tc.tile_pool(name="acc", bufs=1) as ap, \
             tc.tile_pool(name="pmm", bufs=4, space="PSUM") as pmm, \
             tc.tile_pool(name="pdg", bufs=2, space="PSUM") as pdg:

            csh = cp.tile([128, 4, KS], F16)
            cmh = cp.tile([128, 4, KM], F16)
            c2st = cp.tile([2, KS], F16)
            c2mt = cp.tile([2, KM], F16)
            ones2 = cp.tile([2, 128], F16)
            ident_t = cp.tile([128, 128], F32)

            for dc in range(4):
                sl = slice(dc * 128, (dc + 1) * 128)
                nc.sync.dma_start(out=csh[:, dc, :], in_=ins["csT_hi"][sl, :])
                nc.sync.dma_start(out=cmh[:, dc, :], in_=ins["cmT_hi"][sl, :])
            nc.sync.dma_start(out=c2st[:, :], in_=ins["c2s"][:, :])
            nc.sync.dma_start(out=c2mt[:, :], in_=ins["c2m"][:, :])
            nc.sync.dma_start(out=ident_t[:], in_=ins["ident"][:, :])
            nc.vector.memset(ones2[:], 1.0)

            smax_s_b = ap.tile([128, NT], F32)
            smax_m_b = ap.tile([128, NT], F32)
            znf_b = ap.tile([128, NT], F32)
            zns_b = ap.tile([128, NT], F32)
            cnt_s_b = ap.tile([128, NT], F32)
            cnt_m_b = ap.tile([128, NT], F32)
            cmax_s_b = ap.tile([128, NT * 2], F32)
            cmax_m_b = ap.tile([128, NT * 8], F32)

            for t in range(NT):
                tsl = slice(t * 128, (t + 1) * 128)
                zfh = zp.tile([128, 4, 128], F16, name="zfh")
                zsh = zp.tile([128, 4, 128], F16, name="zsh")
                for dc in range(4):
                    dsl = slice(dc * 128, (dc + 1) * 128)
                    nc.sync.dma_start(out=zfh[:, dc, :], in_=ins["zfT_hi"][dsl, tsl])
                    nc.sync.dma_start(out=zsh[:, dc, :], in_=ins["zsT_hi"][dsl, tsl])

                s_syn = sp.tile([128, KS], F32, name="s_syn")
                s_sem = sp.tile([128, KM], F32, name="s_sem")

                for (zh, ch, c2t, s_sb, cmax_b, KB) in (
                        (zfh, csh, c2st, s_syn, cmax_s_b, KS // 512),
                        (zsh, cmh, c2mt, s_sem, cmax_m_b, KM // 512)):
                    for kb in range(KB):
                        fs = slice(kb * 512, (kb + 1) * 512)
                        pt = pmm.tile([128, 512], F32, space="PSUM", name="pt")
                        for dc in range(4):
                            nc.tensor.matmul(out=pt[:], lhsT=zh[:, dc, :],
                                             rhs=ch[:, dc, fs],
                                             start=(dc == 0), stop=False)
                        nc.tensor.matmul(out=pt[:], lhsT=ones2[:, :],
                                         rhs=c2t[:, fs], start=False, stop=True)
                        nc.vector.tensor_scalar(
                            out=s_sb[:, fs], in0=pt[:], scalar1=0.0, scalar2=None,
                            op0=mybir.AluOpType.add, op1=mybir.AluOpType.max,
                            accum_out=cmax_b[:, t * KB + kb: t * KB + kb + 1])

                # ||2z||^2 per latent (hi*hi only; only feeds the loss scalar)
                for (zh, znb) in ((zfh, znf_b), (zsh, zns_b)):
                    dg = pdg.tile([128, 128], F32, space="PSUM", name="dg")
                    for dc in range(4):
                        nc.tensor.matmul(out=dg[:], lhsT=zh[:, dc, :],
                                         rhs=zh[:, dc, :],
                                         start=(dc == 0), stop=(dc == 3))
                    dvec = mp.tile([128, 128], F32, name="dvec")
                    nc.vector.tensor_tensor(out=dvec[:], in0=dg[:], in1=ident_t[:],
                                            op=mybir.AluOpType.mult)
                    nc.vector.reduce_sum(out=znb[:, t:t + 1], in_=dvec[:],
                                         axis=mybir.AxisListType.X)

                for (s_sb, cmax_b, smax_b, cnt_b, KB, Kc, cb_nat, zq_out, idx_out) in (
                        (s_syn, cmax_s_b, smax_s_b, cnt_s_b, 2, KS,
                         ins["cbs_nat"], zq_syn, idx_syn),
                        (s_sem, cmax_m_b, smax_m_b, cnt_m_b, 8, KM,
                         ins["cbm_nat"], zq_sem, idx_sem)):
                    nc.vector.reduce_max(
                        out=smax_b[:, t:t + 1],
                        in_=cmax_b[:, t * KB:(t + 1) * KB],
                        axis=mybir.AxisListType.X)
                    # thrneg = MARGIN - gmax (bias for the Relu ambiguity sum)
                    thrneg = mp.tile([128, 1], F32, name="thrneg")
                    nc.vector.tensor_scalar(
                        out=thrneg[:], in0=smax_b[:, t:t + 1], scalar1=-1.0,
                        scalar2=MARGIN, op0=mybir.AluOpType.mult,
                        op1=mybir.AluOpType.add)
                    junk = jp.tile([128, KM], F32, name="junk")
                    nc.scalar.activation(
                        out=junk[:, :Kc], in_=s_sb[:],
                        func=mybir.ActivationFunctionType.Relu,
                        bias=thrneg[:, 0:1], scale=1.0,
                        accum_out=cnt_b[:, t:t + 1])
                    gmax8 = mp.tile([128, 8], F32, name="gmax8")
                    nc.scalar.activation(
                        out=gmax8[:], in_=smax_b[:, t:t + 1].to_broadcast([128, 8]),
                        func=mybir.ActivationFunctionType.Copy)
                    i8 = mp.tile([128, 8], U32, name="i8")
                    nc.vector.max_index(out=i8[:], in_max=gmax8[:], in_values=s_sb[:])
                    nc.sync.dma_start(out=idx_out[tsl, :],
                                      in_=i8[:, 0:1].bitcast(I32))
                    zqt = qp.tile([128, D], F32, name="zqt")
                    nc.gpsimd.indirect_dma_start(
                        out=zqt[:], out_offset=None, in_=cb_nat[:, :],
                        in_offset=bass.IndirectOffsetOnAxis(
                            ap=i8[:, 0:1].bitcast(I32), axis=0))
                    nc.sync.dma_start(out=zq_out[tsl, :], in_=zqt[:])

            nc.sync.dma_start(out=smax_syn[:, :], in_=smax_s_b[:])
            nc.sync.dma_start(out=smax_sem[:, :], in_=smax_m_b[:])
            nc.sync.dma_start(out=znorm_f[:, :], in_=znf_b[:])
            nc.sync.dma_start(out=znorm_s[:, :], in_=zns_b[:])
            nc.sync.dma_start(out=cnt_syn[:, :], in_=cnt_s_b[:])
            nc.sync.dma_start(out=cnt_sem[:, :], in_=cnt_m_b[:])
            nc.sync.dma_start(out=cmax_syn[:, :], in_=cmax_s_b[:])
            nc.sync.dma_start(out=cmax_sem[:, :], in_=cmax_m_b[:])
    nc.compile()
    return nc


def _build_rescue(plan_s, plan_m):
    """plan_*: tuple of (block_id, n_chunks) with n_chunks >= 1, baked."""
    nc = bacc.Bacc(None, target_bir_lowering=False, debug=False)

    tot_s = sum(nch for _, nch in plan_s)
    tot_m = sum(nch for _, nch in plan_m)
    zin = {}
    if tot_s:
        zin["rzh_s"] = nc.dram_tensor("rzh_s", [D, tot_s * 128], F16,
                                      kind="ExternalInput")
        zin["rzl_s"] = nc.dram_tensor("rzl_s", [D, tot_s * 128], F16,
                                      kind="ExternalInput")
    if tot_m:
        zin["rzh_m"] = nc.dram_tensor("rzh_m", [D, tot_m * 128], F16,
                                      kind="ExternalInput")
        zin["rzl_m"] = nc.dram_tensor("rzl_m", [D, tot_m * 128], F16,
                                      kind="ExternalInput")
    csT_hi = nc.dram_tensor("csT_hi", [D, KS], F16, kind="ExternalInput")
    csT_lo = nc.dram_tensor("csT_lo", [D, KS], F16, kind="ExternalInput")
    cmT_hi = nc.dram_tensor("cmT_hi", [D, KM], F16, kind="ExternalInput")
    cmT_lo = nc.dram_tensor("cmT_lo", [D, KM], F16, kind="ExternalInput")
    c2s = nc.dram_tensor("c2s", [2, KS], F16, kind="ExternalInput")
    c2m = nc.dram_tensor("c2m", [2, KM], F16, kind="ExternalInput")

    outs = {}
    if tot_s:
        outs["rval_s"] = nc.dram_tensor("rval_s", [128, tot_s], F32,
                                        kind="ExternalOutput")
        outs["ridx_s"] = nc.dram_tensor("ridx_s", [128, tot_s], I32,
                                        kind="ExternalOutput")
    if tot_m:
        outs["rval_m"] = nc.dram_tensor("rval_m", [128, tot_m], F32,
                                        kind="ExternalOutput")
        outs["ridx_m"] = nc.dram_tensor("ridx_m", [128, tot_m], I32,
                                        kind="ExternalOutput")

    with tile.TileContext(nc) as tc:
        with tc.tile_pool(name="const", bufs=1) as cp, \
             tc.tile_pool(name="zin", bufs=3) as zp, \
             tc.tile_pool(name="wrk", bufs=3) as wp, \
             tc.tile_pool(name="acc", bufs=1) as ap, \
             tc.tile_pool(name="pmm", bufs=4, space="PSUM") as pmm:

            ones2 = cp.tile([2, 128], F16)
            nc.vector.memset(ones2[:], 1.0)

            for (tag, plan, tot, ch_hi, cl_lo, c2_in, Kc) in (
                    ("s", plan_s, tot_s, csT_hi, csT_lo, c2s, KS),
                    ("m", plan_m, tot_m, cmT_hi, cmT_lo, c2m, KM)):
                if not tot:
                    continue
                blocks = sorted({b for b, _ in plan})
                cbt = {}
                c2t = {}
                for b in blocks:
                    fs = slice(b * 512, (b + 1) * 512)
                    bh = cp.tile([128, 4, 512], F16, name=f"bh_{tag}{b}")
                    bl = cp.tile([128, 4, 512], F16, name=f"bl_{tag}{b}")
                    for dc in range(4):
                        dsl = slice(dc * 128, (dc + 1) * 128)
                        nc.sync.dma_start(out=bh[:, dc, :], in_=ch_hi[dsl, fs])
                        nc.sync.dma_start(out=bl[:, dc, :], in_=cl_lo[dsl, fs])
                    b2 = cp.tile([2, 512], F16, name=f"b2_{tag}{b}")
                    nc.sync.dma_start(out=b2[:, :], in_=c2_in[:, fs])
                    cbt[b] = (bh, bl)
                    c2t[b] = b2

                rvb = ap.tile([128, tot], F32, name=f"rvb_{tag}")
                rib = ap.tile([128, tot], U32, name=f"rib_{tag}")
                gi = 0
                for b, nch in plan:
                    bh, bl = cbt[b]
                    b2 = c2t[b]
                    for _ in range(nch):
                        csl = slice(gi * 128, (gi + 1) * 128)
                        zh = zp.tile([128, 4, 128], F16, name="zh")
                        zl = zp.tile([128, 4, 128], F16, name="zl")
                        for dc in range(4):
                            dsl = slice(dc * 128, (dc + 1) * 128)
                            nc.sync.dma_start(out=zh[:, dc, :],
                                              in_=zin[f"rzh_{tag}"][dsl, csl])
                            nc.sync.dma_start(out=zl[:, dc, :],
                                              in_=zin[f"rzl_{tag}"][dsl, csl])
                        pt = pmm.tile([128, 512], F32, space="PSUM", name="pt")
                        first = True
                        for a_, b_ in ((zh, bh), (zh, bl), (zl, bh)):
                            for dc in range(4):
                                nc.tensor.matmul(out=pt[:], lhsT=a_[:, dc, :],
                                                 rhs=b_[:, dc, :],
                                                 start=first, stop=False)
                                first = False
                        nc.tensor.matmul(out=pt[:], lhsT=ones2[:, :], rhs=b2[:, :],
                                         start=False, stop=True)
                        srow = wp.tile([128, 512], F32, name="srow")
                        nc.vector.tensor_scalar(
                            out=srow[:], in0=pt[:], scalar1=0.0, scalar2=None,
                            op0=mybir.AluOpType.add, op1=mybir.AluOpType.max,
                            accum_out=rvb[:, gi:gi + 1])
                        gmax8 = wp.tile([128, 8], F32, name="gmax8")
                        nc.scalar.activation(
                            out=gmax8[:],
                            in_=rvb[:, gi:gi + 1].to_broadcast([128, 8]),
                            func=mybir.ActivationFunctionType.Copy)
                        i8 = wp.tile([128, 8], U32, name="i8")
                        nc.vector.max_index(out=i8[:], in_max=gmax8[:],
                                            in_values=srow[:])
                        nc.vector.tensor_copy(out=rib[:, gi:gi + 1],
                                              in_=i8[:, 0:1])
                        gi += 1
                nc.sync.dma_start(out=outs[f"rval_{tag}"][:, :], in_=rvb[:])
                nc.sync.dma_start(out=outs[f"ridx_{tag}"][:, :],
                                  in_=rib[:].bitcast(I32))
    nc.compile()
    return nc


def _build_phase2(CH_S, CH_M, MAXC_S, MAXC_M):
    nc = bacc.Bacc(None, target_bir_lowering=False, debug=False)

    zg_s = nc.dram_tensor("zg_s", [CH_S * 128, D], F16, kind="ExternalInput")
    zg_m = nc.dram_tensor("zg_m", [4 * CH_M * 128, D], F16, kind="ExternalInput")
    tgt_s = nc.dram_tensor("tgt_s", [128, CH_S], F32, kind="ExternalInput")
    tgt_m = nc.dram_tensor("tgt_m", [128, 4 * CH_M], F32, kind="ExternalInput")
    avg99_s = nc.dram_tensor("avg99_s", [RS, D], F32, kind="ExternalInput")
    avg99_m = nc.dram_tensor("avg99_m", [RM, D], F32, kind="ExternalInput")
    cl99_s = nc.dram_tensor("cl99_s", [128, 1], F32, kind="ExternalInput")
    cl99_m = nc.dram_tensor("cl99_m", [128, 4], F32, kind="ExternalInput")
    iota_in = nc.dram_tensor("iota_in", [128, 128], BF16, kind="ExternalInput")
    a_s = nc.dram_tensor("a_s", [128, 2], F32, kind="ExternalInput")
    adj_s = nc.dram_tensor("adj_s", [RS, KS], F32, kind="ExternalInput")
    adj_m = nc.dram_tensor("adj_m", [RM, KM], F32, kind="ExternalInput")
    pcol_s = nc.dram_tensor("pcol_s", [128, MAXC_S], I16, kind="ExternalInput")
    pval_s = nc.dram_tensor("pval_s", [128, MAXC_S], BF16, kind="ExternalInput")
    pcol_m = nc.dram_tensor("pcol_m", [4, 4, 128, MAXC_M], I16, kind="ExternalInput")
    pval_m = nc.dram_tensor("pval_m", [4, 4, 128, MAXC_M], BF16, kind="ExternalInput")

    cb_s = nc.dram_tensor("cb_s", [RS, D], F32, kind="ExternalOutput")
    cb_m = nc.dram_tensor("cb_m", [RM, D], F32, kind="ExternalOutput")
    adj_so = nc.dram_tensor("adj_so", [RS, KS], F32, kind="ExternalOutput")
    adj_mo = nc.dram_tensor("adj_mo", [RM, KM], F32, kind="ExternalOutput")

    with tile.TileContext(nc) as tc:
        with tc.tile_pool(name="const", bufs=1) as cp, \
             tc.tile_pool(name="zg", bufs=3) as zp, \
             tc.tile_pool(name="wrk", bufs=3) as wp, \
             tc.tile_pool(name="adj", bufs=3) as adp, \
             tc.tile_pool(name="pme", bufs=2, space="PSUM") as pme, \
             tc.tile_pool(name="pmc", bufs=2, space="PSUM") as pmc:

            iota_t = cp.tile([128, 128], BF16)
            ones1 = cp.tile([128, 1], F16)
            at = cp.tile([128, 2], F32)
            tgt_st = cp.tile([128, CH_S], F32)
            tgt_mt = cp.tile([128, 4 * CH_M], F32)
            cl99_st = cp.tile([128, 1], F32)
            cl99_mt = cp.tile([128, 4], F32)
            nc.sync.dma_start(out=iota_t[:], in_=iota_in[:, :])
            nc.sync.dma_start(out=at[:], in_=a_s[:, :])
            nc.sync.dma_start(out=tgt_st[:], in_=tgt_s[:, :])
            nc.sync.dma_start(out=tgt_mt[:], in_=tgt_m[:, :])
            nc.sync.dma_start(out=cl99_st[:], in_=cl99_s[:, :])
            nc.sync.dma_start(out=cl99_mt[:], in_=cl99_m[:, :])
            nc.vector.memset(ones1[:], 1.0)

            # --- EMA codebook update, segmented one-hot matmuls ---
            for (zg, tgt_t, cl99_t, avg99, cb_o, n_tiles, n_ch, a_col) in (
                    (zg_s, tgt_st, cl99_st, avg99_s, cb_s, 1, CH_S, 0),
                    (zg_m, tgt_mt, cl99_mt, avg99_m, cb_m, 4, CH_M, 1)):
                for ti in range(n_tiles):
                    es = pme.tile([128, 512], F32, space="PSUM", name="es")
                    cnt = pmc.tile([128, 1], F32, space="PSUM", name="cnt")
                    for ch in range(n_ch):
                        gi = ti * n_ch + ch
                        zgt = zp.tile([128, D], F16, name="zgt")
                        nc.sync.dma_start(out=zgt[:],
                                          in_=zg[gi * 128:(gi + 1) * 128, :])
                        oh = wp.tile([128, 128], F16, name="oh")
                        nc.vector.tensor_scalar(
                            out=oh[:], in0=iota_t[:], scalar1=tgt_t[:, gi:gi + 1],
                            scalar2=None, op0=mybir.AluOpType.is_equal)
                        nc.tensor.matmul(out=es[:], lhsT=oh[:], rhs=zgt[:],
                                         start=(ch == 0), stop=(ch == n_ch - 1))
                        nc.tensor.matmul(out=cnt[:], lhsT=oh[:], rhs=ones1[:],
                                         start=(ch == 0), stop=(ch == n_ch - 1))
                    cl_new = wp.tile([128, 1], F32, name="cl_new")
                    nc.vector.tensor_scalar(
                        out=cl_new[:], in0=cnt[:], scalar1=float(ONE_MINUS_DECAY),
                        scalar2=cl99_t[:, ti:ti + 1], op0=mybir.AluOpType.mult,
                        op1=mybir.AluOpType.add)
                    cle = wp.tile([128, 1], F32, name="cle")
                    nc.vector.tensor_scalar_add(out=cle[:], in0=cl_new[:],
                                                scalar1=EPS)
                    rec = wp.tile([128, 1], F32, name="rec")
                    nc.vector.reciprocal(out=rec[:], in_=cle[:])
                    r = wp.tile([128, 1], F32, name="r")
                    nc.vector.tensor_scalar(
                        out=r[:], in0=rec[:], scalar1=at[:, a_col:a_col + 1],
                        scalar2=None, op0=mybir.AluOpType.mult)
                    avgt = wp.tile([128, D], F32, name="avgt")
                    nc.sync.dma_start(out=avgt[:],
                                      in_=avg99[ti * 128:(ti + 1) * 128, :])
                    cbt = wp.tile([128, D], F32, name="cbt")
                    nc.vector.tensor_tensor(out=cbt[:], in0=avgt[:], in1=es[:],
                                            op=mybir.AluOpType.add)
                    nc.vector.tensor_scalar(
                        out=cbt[:], in0=cbt[:], scalar1=r[:, 0:1], scalar2=None,
                        op0=mybir.AluOpType.mult)
                    nc.sync.dma_start(out=cb_o[ti * 128:(ti + 1) * 128, :],
                                      in_=cbt[:])

            # --- adjacency decay update ---
            for (adj_i, adj_o, pcol, pval, n_rt, n_q, maxc, Kc) in (
                    (adj_s, adj_so, pcol_s, pval_s, 1, 1, MAXC_S, KS),
                    (adj_m, adj_mo, pcol_m, pval_m, 4, 4, MAXC_M, KM)):
                for rt in range(n_rt):
                    for q in range(n_q):
                        csl = slice(q * QS, (q + 1) * QS)
                        colt = adp.tile([128, maxc], I16, name="colt")
                        valt = adp.tile([128, maxc], BF16, name="valt")
                        if n_q == 1:
                            nc.sync.dma_start(out=colt[:], in_=pcol[:, :])
                            nc.sync.dma_start(out=valt[:], in_=pval[:, :])
                        else:
                            nc.sync.dma_start(out=colt[:], in_=pcol[rt, q, :, :])
                            nc.sync.dma_start(out=valt[:], in_=pval[rt, q, :, :])
                        pcd = adp.tile([128, QS], BF16, name="pcd")
                        nc.gpsimd.local_scatter(
                            out_ap=pcd[:], data_ap=valt[:], idxs_ap=colt[:],
                            channels=128, num_elems=QS, num_idxs=maxc)
                        dp = adp.tile([128, QS], F32, name="dp")
                        nc.scalar.activation(
                            out=dp[:], in_=pcd[:],
                            func=mybir.ActivationFunctionType.Exp,
                            scale=LN_ADJ_DECAY)
                        adjt = adp.tile([128, QS], F32, name="adjt")
                        nc.sync.dma_start(out=adjt[:],
                                          in_=adj_i[rt * 128:(rt + 1) * 128, csl])
                        u = adp.tile([128, QS], F32, name="u")
                        nc.vector.scalar_tensor_tensor(
                            out=u[:], in0=adjt[:], scalar=-ADJ_Q, in1=dp[:],
                            op0=mybir.AluOpType.add, op1=mybir.AluOpType.mult)
                        nc.vector.tensor_scalar_add(out=u[:], in0=u[:],
                                                    scalar1=ADJ_Q)
                        nc.sync.dma_start(out=adj_o[rt * 128:(rt + 1) * 128, csl],
                                          in_=u[:])
    nc.compile()
    return nc


def _split16(x):
    hi = x.astype(np.float16)
    lo = (x - hi.astype(np.float32)).astype(np.float16)
    return hi, lo


def _pair_meta(prev_loc, cols, n_rows, n_q, maxc, name):
    """Per-(row, quarter) unique idx columns + counts, padded to maxc."""
    n_rt = n_rows // 128
    pcol = np.full((n_rt, n_q, 128, maxc), -1, np.int16)
    pval = np.zeros((n_rt, n_q, 128, maxc), np.float32)
    if len(prev_loc):
        key = prev_loc.astype(np.int64) * (n_q * QS) + cols.astype(np.int64)
        uniq, cnts = np.unique(key, return_counts=True)
        r = (uniq // (n_q * QS)).astype(np.int64)
        cc = (uniq % (n_q * QS)).astype(np.int64)
        qq = cc // QS
        cloc = cc % QS
        rt = r // 128
        rloc = r % 128
        bucket = ((rt * n_q + qq) * 128 + rloc)
        order = np.argsort(bucket, kind="stable")
        bucket_s = bucket[order]
        starts = np.searchsorted(bucket_s, bucket_s, side="left")
        pos = np.arange(len(bucket_s)) - starts
        if len(pos) and pos.max() >= maxc:
            raise RuntimeError(f"{name}: pair list overflow {pos.max()+1} > {maxc}")
        pcol[rt[order], qq[order], rloc[order], pos] = cloc[order].astype(np.int16)
        pval[rt[order], qq[order], rloc[order], pos] = cnts[order]
    return pcol, pval


def kernel(**inputs):
    global _PHASE1_NC
    import ml_dtypes

    core_ids = list(range(C))

    # ---------------- phase 1 host prep ----------------
    zf = np.concatenate([np.asarray(inputs["z_fast_real"], np.float32),
                         np.asarray(inputs["z_fast_imag"], np.float32)], axis=1)
    zs = np.concatenate([np.asarray(inputs["z_slow_real"], np.float32),
                         np.asarray(inputs["z_slow_imag"], np.float32)], axis=1)
    cbs = np.ascontiguousarray(np.asarray(inputs["cb_syn"], np.float32))
    cbm = np.ascontiguousarray(np.asarray(inputs["cb_sem"], np.float32))

    zf2_hi, zf2_lo = _split16(zf * np.float32(2.0))
    zs2_hi, zs2_lo = _split16(zs * np.float32(2.0))
    zfT_hi = np.ascontiguousarray(zf2_hi.T)
    zfT_lo = np.ascontiguousarray(zf2_lo.T)
    zsT_hi = np.ascontiguousarray(zs2_hi.T)
    zsT_lo = np.ascontiguousarray(zs2_lo.T)

    cs_hi, cs_lo = _split16(cbs)
    cm_hi, cm_lo = _split16(cbm)
    csT_hi = np.ascontiguousarray(cs_hi.T)
    csT_lo = np.ascontiguousarray(cs_lo.T)
    cmT_hi = np.ascontiguousarray(cm_hi.T)
    cmT_lo = np.ascontiguousarray(cm_lo.T)

    def c2rows(cb):
        c2 = -(cb.astype(np.float64) ** 2).sum(1)
        hi = c2.astype(np.float16)
        lo = (c2 - hi.astype(np.float64)).astype(np.float16)
        return np.ascontiguousarray(np.stack([hi, lo]))

    c2s = c2rows(cbs)
    c2m = c2rows(cbm)
    ident = np.eye(128, dtype=np.float32)

    shared1 = {"csT_hi": csT_hi, "cmT_hi": cmT_hi, "c2s": c2s, "c2m": c2m,
               "cbs_nat": cbs, "cbm_nat": cbm, "ident": ident}
    in_maps1 = []
    for c in range(C):
        sl = slice(c * NLAT, (c + 1) * NLAT)
        m = dict(shared1)
        m["zfT_hi"] = np.ascontiguousarray(zfT_hi[:, sl])
        m["zsT_hi"] = np.ascontiguousarray(zsT_hi[:, sl])
        in_maps1.append(m)

    if _PHASE1_NC is None:
        _PHASE1_NC = _build_phase1()
    res1 = _run(_PHASE1_NC, in_maps1, core_ids, "phase1")
    r1 = res1.results

    zq_syn = np.concatenate([r["zq_syn"] for r in r1], axis=0)
    zq_sem = np.concatenate([r["zq_sem"] for r in r1], axis=0)
    idx_syn = np.concatenate([r["idx_syn"][:, 0] for r in r1]).astype(np.int64)
    idx_sem = np.concatenate([r["idx_sem"][:, 0] for r in r1]).astype(np.int64)

    # [N] views of the per-core [128, NT] stat buffers: n = c*NLAT + t*128 + p
    def flat_stats(key):
        return np.concatenate([r[key].T.reshape(-1) for r in r1])

    smax_s = flat_stats("smax_syn")
    smax_m = flat_stats("smax_sem")
    cnt_s = flat_stats("cnt_syn")
    cnt_m = flat_stats("cnt_sem")
    cmax_s = np.concatenate(
        [r["cmax_syn"].T.reshape(NT, 2, 128).transpose(0, 2, 1).reshape(-1, 2)
         for r in r1])
    cmax_m = np.concatenate(
        [r["cmax_sem"].T.reshape(NT, 8, 128).transpose(0, 2, 1).reshape(-1, 8)
         for r in r1])

    # ---------------- rescue of ambiguous rows ----------------
    def ambiguous(smax, cnt):
        thrneg = (smax * np.float32(-1.0) + np.float32(MARGIN)).astype(np.float32)
        own = np.maximum(np.float32(0.0), smax + thrneg)
        return (cnt - own) > 1e-3

    amb_s = ambiguous(smax_s, cnt_s)
    amb_m = ambiguous(smax_m, cnt_m)

    def rescue_plan(amb, cmax, smax, KB):
        rows = np.nonzero(amb)[0]
        per_core_rows = [[] for _ in range(C)]
        for i, row in enumerate(rows):
            per_core_rows[i % C].append(row)
        buckets = [[[] for _ in range(KB)] for _ in range(C)]
        for c in range(C):
            for row in per_core_rows[c]:
                cand = np.nonzero(cmax[row] >= smax[row] - np.float32(MARGIN))[0]
                for b in cand:
                    buckets[c][b].append(row)
        plan = []
        for b in range(KB):
            mx = max(int(np.ceil(len(buckets[c][b]) / 128.0)) for c in range(C))
            if mx > 0:
                plan.append((b, mx))
        return rows, tuple(plan), buckets

    rows_s, plan_s, buck_s = rescue_plan(amb_s, cmax_s, smax_s, 2)
    rows_m, plan_m, buck_m = rescue_plan(amb_m, cmax_m, smax_m, 8)

    if len(rows_s) or len(rows_m):
        key_r = (plan_s, plan_m)
        if key_r not in _RESCUE_NC:
            _RESCUE_NC[key_r] = _build_rescue(plan_s, plan_m)

        def rescue_inputs(plan, buckets, zT_hi, zT_lo, c):
            tot = sum(nch for _, nch in plan)
            zh = np.zeros((D, tot * 128), np.float16)
            zl = np.zeros((D, tot * 128), np.float16)
            rowmap = np.full((tot * 128,), -1, np.int64)
            gi = 0
            for b, nch in plan:
                rows_cb = buckets[c][b]
                pos = gi * 128
                if rows_cb:
                    zh[:, pos:pos + len(rows_cb)] = zT_hi[:, rows_cb]
                    zl[:, pos:pos + len(rows_cb)] = zT_lo[:, rows_cb]
                    rowmap[pos:pos + len(rows_cb)] = rows_cb
                gi += nch
            return zh, zl, rowmap

        in_maps_r = []
        rowmaps = []
        for c in range(C):
            m = {"csT_hi": csT_hi, "csT_lo": csT_lo, "cmT_hi": cmT_hi,
                 "cmT_lo": cmT_lo, "c2s": c2s, "c2m": c2m}
            rm = {}
            if sum(n for _, n in plan_s):
                zh, zl, rmap = rescue_inputs(plan_s, buck_s, zfT_hi, zfT_lo, c)
                m["rzh_s"] = zh
                m["rzl_s"] = zl
                rm["s"] = rmap
            if sum(n for _, n in plan_m):
                zh, zl, rmap = rescue_inputs(plan_m, buck_m, zsT_hi, zsT_lo, c)
                m["rzh_m"] = zh
                m["rzl_m"] = zl
                rm["m"] = rmap
            in_maps_r.append(m)
            rowmaps.append(rm)

        res_r = _run(_RESCUE_NC[key_r], in_maps_r, core_ids, "rescue")
        rr = res_r.results

        def merge(plan, tag, idx, smax, zq, cb_nat, rows):
            if not sum(n for _, n in plan):
                return
            best_val = {}
            best_idx = {}
            for c in range(C):
                rv = rr[c][f"rval_{tag}"]
                ri = rr[c][f"ridx_{tag}"]
                rmap = rowmaps[c][tag]
                gi = 0
                for b, nch in plan:
                    for _ in range(nch):
                        colbase = gi * 128
                        for p in range(128):
                            row = rmap[colbase + p]
                            if row < 0:
                                continue
                            v = rv[p, gi]
                            gidx = b * 512 + int(ri[p, gi])
                            if (row not in best_val or v > best_val[row]
                                    or (v == best_val[row]
                                        and gidx < best_idx[row])):
                                best_val[row] = v
                                best_idx[row] = gidx
                        gi += 1
            for row in rows:
                v = best_val[row]
                gidx = best_idx[row]
                smax[row] = v
                if idx[row] != gidx:
                    idx[row] = gidx
                    zq[row] = cb_nat[gidx]

        merge(plan_s, "s", idx_syn, smax_s, zq_syn, cbs, rows_s)
        merge(plan_m, "m", idx_sem, smax_m, zq_sem, cbm, rows_m)

    znf_sum = np.float64(sum(r["znorm_f"].astype(np.float64).sum() for r in r1))
    zns_sum = np.float64(sum(r["znorm_s"].astype(np.float64).sum() for r in r1))
    dmin_s = znf_sum / 4.0 - smax_s.astype(np.float64).sum()
    dmin_m = zns_sum / 4.0 - smax_m.astype(np.float64).sum()
    loss = np.float32(1.25 * (dmin_s + dmin_m) / (N * D))
    idx_syn = idx_syn.astype(np.int32)
    idx_sem = idx_sem.astype(np.int32)

    # ---------------- phase 2 host prep ----------------
    prev_syn = np.asarray(inputs["prev_idx_syn"]).astype(np.int64)
    prev_sem = np.asarray(inputs["prev_idx_sem"]).astype(np.int64)
    cl_syn = np.asarray(inputs["cl_syn"], np.float32)
    cl_sem = np.asarray(inputs["cl_sem"], np.float32)
    avg_syn = np.asarray(inputs["avg_syn"], np.float32)
    avg_sem = np.asarray(inputs["avg_sem"], np.float32)
    adj_syn = np.asarray(inputs["adj_syn"], np.float32)
    adj_sem = np.asarray(inputs["adj_sem"], np.float32)

    zf01 = (zf * ONE_MINUS_DECAY).astype(np.float16)
    zs01 = (zs * ONE_MINUS_DECAY).astype(np.float16)

    def chunks_needed(idx, K):
        counts = np.bincount(idx, minlength=K)
        tc_ = counts.reshape(K // 128, 128).sum(1)
        return max(1, int(np.ceil(tc_.max() / 128.0)))

    CH_S = chunks_needed(idx_syn, KS)
    CH_M = chunks_needed(idx_sem, KM)

    def maxc_needed(prev, idx, n_q):
        key = prev.astype(np.int64) * (n_q * QS) + idx.astype(np.int64)
        uniq = np.unique(key)
        rq = uniq // QS
        _, cnt = np.unique(rq, return_counts=True)
        m = int(cnt.max()) if len(cnt) else 2
        return (m + 1) // 2 * 2

    MAXC_S = maxc_needed(prev_syn, idx_syn.astype(np.int64), 1)
    MAXC_M = maxc_needed(prev_sem, idx_sem.astype(np.int64), 4)

    def embed_meta(idx, z01, K, n_tiles, n_ch, name):
        order = np.argsort(idx, kind="stable")
        counts = np.bincount(idx, minlength=K)
        cum = np.concatenate([[0], np.cumsum(counts)])
        zg_all, tgt_all = [], []
        rows_per_tile = 128 * n_ch
        for c in range(C):
            lo_code = c * (K // C)
            zg = np.zeros((n_tiles * rows_per_tile, D), np.float16)
            tgt = np.full((n_tiles * n_ch * 128,), -1.0, np.float32)
            for ti in range(n_tiles):
                base = lo_code + ti * 128
                lat = order[cum[base]:cum[base + 128]]
                if len(lat) > rows_per_tile:
                    raise RuntimeError(f"{name}: tile overflow {len(lat)}")
                zg[ti * rows_per_tile: ti * rows_per_tile + len(lat)] = z01[lat]
                tgt[ti * rows_per_tile: ti * rows_per_tile + len(lat)] = \
                    (idx[lat] - base).astype(np.float32)
            tgt2 = np.ascontiguousarray(tgt.reshape(n_tiles * n_ch, 128).T)
            zg_all.append(zg)
            tgt_all.append(tgt2)
        return zg_all, tgt_all

    zg_s_all, tgt_s_all = embed_meta(idx_syn, zf01, KS, 1, CH_S, "syn")
    zg_m_all, tgt_m_all = embed_meta(idx_sem, zs01, KM, 4, CH_M, "sem")

    n_syn = 0.99 * cl_syn.astype(np.float64).sum() + 0.01 * N
    n_sem = 0.99 * cl_sem.astype(np.float64).sum() + 0.01 * N
    A_syn = float((n_syn + KS * EPS) / n_syn)
    A_sem = float((n_sem + KM * EPS) / n_sem)
    a_arr = np.ascontiguousarray(np.broadcast_to(
        np.array([[A_syn, A_sem]], np.float32), (128, 2)))
    iota_bc = np.broadcast_to(np.arange(128, dtype=np.float32), (128, 128))
    iota_bc = np.ascontiguousarray(iota_bc).astype(ml_dtypes.bfloat16)

    avg99_s_full = avg_syn * DECAY
    avg99_m_full = avg_sem * DECAY
    cl99_s_full = cl_syn * DECAY
    cl99_m_full = cl_sem * DECAY

    in_maps2 = []
    for c in range(C):
        ssl = slice(c * RS, (c + 1) * RS)
        msl = slice(c * RM, (c + 1) * RM)
        sel_s = (prev_syn // RS) == c
        sel_m = (prev_sem // RM) == c
        pcol_s, pval_s = _pair_meta(prev_syn[sel_s] - c * RS,
                                    idx_syn[sel_s].astype(np.int64),
                                    RS, 1, MAXC_S, "syn-pair")
        pcol_m, pval_m = _pair_meta(prev_sem[sel_m] - c * RM,
                                    idx_sem[sel_m].astype(np.int64),
                                    RM, 4, MAXC_M, "sem-pair")
        m = {
            "zg_s": zg_s_all[c], "zg_m": zg_m_all[c],
            "tgt_s": tgt_s_all[c], "tgt_m": tgt_m_all[c],
            "avg99_s": np.ascontiguousarray(avg99_s_full[ssl]),
            "avg99_m": np.ascontiguousarray(avg99_m_full[msl]),
            "cl99_s": np.ascontiguousarray(cl99_s_full[ssl].reshape(128, 1)),
            "cl99_m": np.ascontiguousarray(cl99_m_full[msl].reshape(4, 128).T),
            "iota_in": iota_bc, "a_s": a_arr,
            "adj_s": np.ascontiguousarray(adj_syn[ssl]),
            "adj_m": np.ascontiguousarray(adj_sem[msl]),
            "pcol_s": pcol_s[0, 0], "pval_s": pval_s[0, 0].astype(ml_dtypes.bfloat16),
            "pcol_m": pcol_m, "pval_m": pval_m.astype(ml_dtypes.bfloat16),
        }
        in_maps2.append(m)

    key2 = (CH_S, CH_M, MAXC_S, MAXC_M)
    if key2 not in _PHASE2_NC:
        _PHASE2_NC[key2] = _build_phase2(*key2)
    res2 = _run(_PHASE2_NC[key2], in_maps2, core_ids, "phase2")
    r2 = res2.results

    cb_syn_new = np.concatenate([r["cb_s"] for r in r2], axis=0)
    cb_sem_new = np.concatenate([r["cb_m"] for r in r2], axis=0)
    adj_syn_new = np.concatenate([r["adj_so"] for r in r2], axis=0)
    adj_sem_new = np.concatenate([r["adj_mo"] for r in r2], axis=0)

    return (zq_syn, zq_sem, loss, idx_syn, idx_sem,
            cb_syn_new, cb_sem_new, adj_syn_new, adj_sem_new)
